# revision 84
# baseline (speedup 1.0000x reference)
"""Causal MHA with RoPE on 8 TRN2 NeuronCores.

Sharding: data-parallel over batch (2) x tensor-parallel over heads (4 groups
of 4 heads) = 8 cores. Core c handles batch c//4, head group c%4.
Each core computes its 4 heads' attention and a partial output projection
(Wo sharded row-wise); host sums the 4 partials per batch.

Per-core device algorithm (all 2-byte operands are fp16; PSUM f32):
  - QK^T projection: qkT[dk, s] = (Wqk rows).T-contracted with xT (fp16),
    RoPE applied on [dk(partition), s] layout via cos/sin tables and a
    stream_shuffle pair swap, output fp16
  - V projection: vt[k, head, dk+1] fp16 with a ones column for softmax sums
  - scores^T[k, q] = K^T.T-free @ Q^T per head (K=64 contraction, two heads
    in row groups 0-1 / 2-3)
  - probsT = exp(scores/8) fp16 straight from PSUM (no max subtraction;
    scores are N(0,1)-scaled), causal tri-mask on diagonal tiles (DVE 2x)
  - PV in q-partition layout: attn[q, dk+1] += probsT[:, qsub].T @ [V|1]
    per 128-q block; 65-wide matmuls fill all 128 output partitions.
    Chains are emitted contiguously per PSUM region: interleaved
    accumulation groups in one PSUM bank are silently broken on HW.
  - softmax normalization fused into the PSUM->SBUF copy (DVE
    tensor_scalar_mul with a per-partition reciprocal)
  - attn[q, feat] -> attnT[feat, q] via xbar DMA transpose (fp16), or PE
    transpose on the drain path
  - partial out = attnT.T-contracted with WoT, DMA'd to DRAM

Schedule: one fused loop over s-tiles t: projection(t) -> attention(qt=t)
(causal: qt only needs k <= (t+1)*512). The V-projection chains and the
previous qt's transposes/out-projection are sliced into the kb loop as PE
fill work, because the kb loop alone is ACT(exp)-bound and ss PSUM
double-buffering stalls scores otherwise. DMAs are consolidated (each op
costs ~625ns serial HWDGE time) and ordered by first use.
"""
import sys
import os

for _p in ("/opt/trn_rl_repo", "/root/.axon_site/_ro/trn_rl_repo"):
    if os.path.isdir(_p) and _p not in sys.path:
        sys.path.insert(0, _p)

import numpy as np

import concourse.mybir as mybir
import concourse.tile as tile
from concourse import bacc
from concourse.bass_utils import run_bass_kernel_spmd

F32 = mybir.dt.float32
F32R = mybir.dt.float32r
F16 = mybir.dt.float16
AF = mybir.ActivationFunctionType
MULT = mybir.AluOpType.mult
ADD = mybir.AluOpType.add

B, S, D = 2, 2048, 1024
H, DK = 16, 64
THETA = 10000.0
NCORES = 8
GROUPS = 4          # head groups (tensor parallel)
GH = H // GROUPS    # heads per group = 4
GF = GH * DK        # features per group = 256
SWAP_MASK = [i ^ 1 for i in range(32)]
KVER = 41  # bump on any kernel change: busts the HLO-shape-keyed NEFF cache

_CACHED = {}


def _build_nc(iters=1):
    nc = bacc.Bacc("TRN2", target_bir_lowering=False, debug=False, num_devices=NCORES)
    xT = nc.dram_tensor("xT", [D, S], F16, kind="ExternalInput").ap()
    wqkT = nc.dram_tensor("wqkT", [D, 2 * GF], F16, kind="ExternalInput").ap()
    wvT = nc.dram_tensor("wvT", [D, GF], F16, kind="ExternalInput").ap()
    woT = nc.dram_tensor("woT", [GF, D], F16, kind="ExternalInput").ap()
    cosf = nc.dram_tensor("cosf", [128, S], F16, kind="ExternalInput").ap()
    sins = nc.dram_tensor("sins", [128, S], F16, kind="ExternalInput").ap()
    tri = nc.dram_tensor("tri", [128, 128], F16, kind="ExternalInput").ap()
    iden = nc.dram_tensor("iden", [128, 128], F16, kind="ExternalInput").ap()
    # unused input whose shape encodes the kernel version: the neuron compile
    # cache keys on HLO structure only, so two kernels with identical I/O
    # shapes would otherwise collide.
    nc.dram_tensor("cachebust", [iters, KVER], F32, kind="ExternalInput")
    out = nc.dram_tensor("out", [S, D], F32, kind="ExternalOutput").ap()

    SB = S // 512  # 4 q-tiles of 512
    KB = S // 128  # 16 k-blocks of 128

    with tile.TileContext(nc) as tc:
        with tc.tile_pool(name="const", bufs=1) as cpool, \
             tc.tile_pool(name="xt", bufs=1) as xpool, \
             tc.tile_pool(name="big", bufs=1) as bpool, \
             tc.tile_pool(name="work", bufs=3) as wpool, \
             tc.tile_pool(name="attn", bufs=18) as apool, \
             tc.tile_pool(name="rcp", bufs=4) as rpool, \
             tc.tile_pool(name="osb", bufs=4) as opool, \
             tc.tile_pool(name="probs", bufs=34) as ppool, \
             tc.tile_pool(name="psum", bufs=1, space="PSUM") as psum:

            # ---- loads, ordered by first use on the single HWDGE queue ----
            wqk_sb = cpool.tile([128, 8, 2 * GF], F16, tag="wqk")
            wv_sb = cpool.tile([128, 8, GF], F16, tag="wv")
            wo_sb = cpool.tile([128, 2, D], F16, tag="wo")
            cos_sb = cpool.tile([128, S], F16, tag="cos")
            sin_sb = cpool.tile([128, S], F16, tag="sin")
            tri_sb = cpool.tile([128, 128], F16, tag="tri")
            iden_sb = cpool.tile([128, 128], F16, tag="iden")
            xt_all = xpool.tile([128, 8, S], F16, tag="xt")

            # consolidated DMAs: each costs ~625ns serial HWDGE time, so
            # fewer+bigger ops beat per-dc chunks
            xdram = xT.rearrange("(dc p) s -> p dc s", p=128)
            wqkd = wqkT.rearrange("(dc p) n -> p dc n", p=128)
            nc.sync.dma_start(wqk_sb[:, 0:4, 0:128], wqkd[:, 0:4, 0:128])
            nc.sync.dma_start(xt_all[:, 0:4, 0:512], xdram[:, 0:4, 0:512])
            nc.sync.dma_start(wqk_sb[:, 4:8, 0:128], wqkd[:, 4:8, 0:128])
            nc.sync.dma_start(xt_all[:, 4:8, 0:512], xdram[:, 4:8, 0:512])
            nc.sync.dma_start(wqk_sb[:, :, 128:512], wqkd[:, :, 128:512])
            nc.sync.dma_start(wv_sb[:], wvT.rearrange("(dc p) n -> p dc n", p=128))
            nc.sync.dma_start(cos_sb[:, 0:1024], cosf[:, 0:1024])
            nc.sync.dma_start(sin_sb[:, 0:1024], sins[:, 0:1024])
            nc.sync.dma_start(xt_all[:, :, 512:1024], xdram[:, :, 512:1024])
            nc.sync.dma_start(tri_sb[:], tri)
            nc.sync.dma_start(iden_sb[:], iden)
            nc.sync.dma_start(xt_all[:, :, 1024:S], xdram[:, :, 1024:S])
            nc.sync.dma_start(cos_sb[:, 1024:S], cosf[:, 1024:S])
            nc.sync.dma_start(sin_sb[:, 1024:S], sins[:, 1024:S])
            nc.sync.dma_start(wo_sb[:], woT.rearrange("(fc p) n -> p fc n", p=128))

            warm = cpool.tile([1, 1], F32, tag="warm")
            nc.scalar.activation(warm[:], cos_sb[0:1, 0:1], AF.Exp, scale=1.0)

            # qkT slabs: 0,1 = Q head-pairs; 2,3 = K head-pairs
            for _it in range(iters):
              qkT = bpool.tile([128, 4, S], F16, tag="qkT", name=f"qkT{_it}")
              vt = bpool.tile([128, KB, GH, DK + 1], F16, tag="vt", name=f"vt{_it}")
              nc.vector.memset(vt[:, :, :, DK:DK + 1], 1.0)
              attnT = [cpool.tile([128, S], F16, tag=f"attnT{p}",
                                  name=f"attnT{p}_{_it}") for p in range(2)]

              def proj_pieces(t):
                  # QK projection + RoPE + V projection for s-range
                  # [t*512, (t+1)*512). QK chunks are returned as `up` (must
                  # run before attention t); V chunks as `vp` — they only
                  # write vt, which attention t reads only in its end-of-loop
                  # PV chains, so they can fill the kb loop.
                  up, vp = [], []
                  # chain order Q0,K0,Q1,K1: pair 0's scores only need slabs
                  # 0 and 2, so they unblock after two chains, not three
                  for c in (0, 2, 1, 3):
                      def qk_piece(c=c):
                          ps = psum.tile([128, 512], F32, tag="sc", bufs=2)
                          for dc in range(8):
                              nc.tensor.matmul(
                                  ps[:], wqk_sb[:, dc, c * 128:(c + 1) * 128],
                                  xt_all[:, dc, t * 512:(t + 1) * 512],
                                  start=(dc == 0), stop=(dc == 7))
                          tsl = slice(t * 512, (t + 1) * 512)
                          # rope: qkT = ps*cos + swap(ps*sins)
                          nc.vector.tensor_tensor(qkT[:, c, tsl], ps[:],
                                                  cos_sb[:, tsl], MULT)
                          tmp = wpool.tile([128, 512], F16, tag="ropetmp")
                          nc.vector.tensor_tensor(tmp[:], ps[:], sin_sb[:, tsl], MULT)
                          tmp2 = wpool.tile([128, 512], F16, tag="ropetmp2")
                          nc.vector.stream_shuffle(tmp2[:], tmp[:], SWAP_MASK)
                          nc.gpsimd.tensor_tensor(qkT[:, c, tsl], qkT[:, c, tsl],
                                                  tmp2[:], ADD)
                      up.append(qk_piece)
                  for sb_i in range(4 * t, 4 * t + 4):
                      def v_piece(sb_i=sb_i):
                          psv = psum.tile([128, GF], F32, tag="sc", bufs=2)
                          for dc in range(8):
                              nc.tensor.matmul(
                                  psv[:], xt_all[:, dc, sb_i * 128:(sb_i + 1) * 128],
                                  wv_sb[:, dc, :], start=(dc == 0), stop=(dc == 7))
                          nc.vector.tensor_copy(
                              vt[:, sb_i, :, 0:DK],
                              psv[:].rearrange("p (h d) -> p h d", h=GH))
                      vp.append(v_piece)
                  return up, vp

              def tail_pieces(qt, at_tiles, last=False):
                  # deferred per-qt tail as small closures so the driver can
                  # interleave them into the next qt's kb loop (fills PE
                  # during exp waits); ordered tr,tr,proj,proj per qb-pair so
                  # output DMAs start as early as possible
                  copy_fn = nc.scalar.copy if last else nc.vector.tensor_copy
                  pieces = []
                  osb2 = [None, None]

                  def tr_piece(j):
                      # xbar DMA transpose: no PE/PSUM use, 2-byte dtype only.
                      # The final tail uses the lower-latency PE transpose
                      # instead (it sits on the kernel's drain path); its
                      # pair-0 transposes were already emitted inside
                      # emit_attn right after pair 0's normalization.
                      for pair in ((1,) if last else (0, 1)):
                          qb = 4 * qt + j
                          qsl2 = slice(qb * 128, (qb + 1) * 128)
                          if last:
                              tr = psum.tile([128, 128], F16, tag="sc", bufs=2,
                                             name=f"tr{qt}_{pair}_{j}_{_it}")
                              nc.tensor.transpose(tr[:], at_tiles[(pair, j)][:],
                                                  iden_sb[:])
                              nc.vector.tensor_copy(attnT[pair][:, qsl2], tr[:])
                          else:
                              nc.sync.dma_start_transpose(attnT[pair][:, qsl2],
                                                          at_tiles[(pair, j)][:])

                  def proj_piece(jh, jj, nh):
                      qb = 4 * qt + 2 * jh + jj
                      qsl = slice(qb * 128, (qb + 1) * 128)
                      nsl = slice(nh * 512, (nh + 1) * 512)
                      pso = psum.tile([128, 512], F32, tag="sc", bufs=2)
                      nc.tensor.matmul(pso[:], attnT[0][:, qsl],
                                       wo_sb[:, 0, nsl], start=True, stop=False)
                      nc.tensor.matmul(pso[:], attnT[1][:, qsl],
                                       wo_sb[:, 1, nsl], start=False, stop=True)
                      if osb2[jh] is None:
                          osb2[jh] = opool.tile([128, 2, D], F32, tag="osb",
                                                name=f"osb{qt}_{jh}_{_it}")
                      copy_fn(osb2[jh][:, jj, nsl], pso[:])
                      if nh == 1 and last:
                          # per-qb DMA at the drain: start output sooner
                          nc.sync.dma_start(out[qsl, :], osb2[jh][:, jj, :])
                      elif nh == 1 and jj == 1:
                          qb0 = 4 * qt + 2 * jh
                          dsl = out[qb0 * 128:(qb0 + 2) * 128, :]
                          nc.sync.dma_start(
                              dsl.rearrange("(two p) n -> p two n", p=128),
                              osb2[jh][:])

                  for jh in range(2):
                      pieces.append(lambda jh=jh: tr_piece(2 * jh))
                      pieces.append(lambda jh=jh: tr_piece(2 * jh + 1))
                      for jj in range(2):
                          for nh in range(2):
                              pieces.append(
                                  lambda jh=jh, jj=jj, nh=nh: proj_piece(jh, jj, nh))
                  return pieces

              def emit_attn(qt, pieces, pvq, last=False):
                  # pvq: pending PV/norm closures from the previous head-pair
                  # (possibly of the previous qt) — popped as fill, always
                  # ahead of `pieces` so normalization is emitted before the
                  # deferred transposes that consume it
                  at_tiles = {}
                  for pair in range(2):
                      qs, ks = pair, 2 + pair
                      pva = psum.tile([128, 4, DK + 1], F32, tag="pva", bufs=1,
                                      name=f"pva{qt}_{pair}_{_it}")
                      pvb = psum.tile([128, 4, DK + 1], F32, tag="pvb", bufs=1,
                                      name=f"pvb{qt}_{pair}_{_it}")
                      nkb = 4 * qt + 4
                      pabs = []
                      for kb in range(nkb):
                          j0 = max(kb - 4 * qt, 0)
                          lam = j0 * 128
                          qsl = slice(qt * 512 + lam, (qt + 1) * 512)
                          ksl = slice(kb * 128, (kb + 1) * 128)
                          ss = psum.tile([128, 2, 512], F32, tag="sc2", bufs=2)
                          nc.tensor.matmul(ss[:, 0, lam:512], qkT[0:64, ks, ksl],
                                           qkT[0:64, qs, qsl], start=True, stop=True)
                          nc.tensor.matmul(ss[:, 1, lam:512], qkT[64:128, ks, ksl],
                                           qkT[64:128, qs, qsl], start=True, stop=True)
                          pab = ppool.tile([128, 2, 512], F16, tag="probs")
                          nc.scalar.activation(pab[:, :, lam:512], ss[:, :, lam:512],
                                               AF.Exp, scale=0.125)
                          if kb >= 4 * qt:  # diagonal block: causal tri mask
                              dsl = slice(lam, lam + 128)
                              nc.vector.tensor_tensor(
                                  pab[:, :, dsl], pab[:, :, dsl],
                                  tri_sb[:, None, :].to_broadcast([128, 2, 128]), MULT)
                          pabs.append(pab)
                          # drain fill work at a rate that empties it by the
                          # end of the loop (leftovers flushed later fill
                          # nothing)
                          backlog = len(pvq) + len(pieces)
                          pops = max(1, -(-backlog // (nkb - kb)))
                          for _ in range(min(pops, backlog)):
                              if pvq:
                                  pvq.pop(0)()
                              elif pieces:
                                  pieces.pop(0)()

                      def pv_norm(h2, pv, pair=pair, pabs=pabs):
                          # PV chains stay contiguous per PSUM region:
                          # interleaved accumulation groups in one PSUM bank
                          # are broken on HW
                          for j in range(4):
                              for kk in range(4 * qt + j + 1):
                                  nc.tensor.matmul(
                                      pv[:, j, :],
                                      pabs[kk][:, h2, j * 128:(j + 1) * 128],
                                      vt[:, kk, 2 * pair + h2, :],
                                      start=(kk == 0), stop=(kk == 4 * qt + j))
                          rc = rpool.tile([128, 4, 1], F32, tag="rc")
                          nc.vector.reciprocal(rc[:], pv[:, :, DK:DK + 1])
                          for j in range(4):
                              key = (pair, j)
                              if key not in at_tiles:
                                  at_tiles[key] = apool.tile(
                                      [128, 128], F16, tag="at",
                                      name=f"at{qt}_{pair}_{j}_{_it}")
                              nc.vector.tensor_scalar_mul(
                                  at_tiles[key][:, h2 * 64:(h2 + 1) * 64],
                                  pv[:, j, 0:DK], rc[:, j, :])

                      def early_tr():
                          # drain-path shortening for the last qt: transpose
                          # pair 0 while pair 1 still computes
                          for j in range(4):
                              qb = 4 * qt + j
                              qsl2 = slice(qb * 128, (qb + 1) * 128)
                              tr = psum.tile([128, 128], F16, tag="sc", bufs=2,
                                             name=f"etr{qt}_{j}_{_it}")
                              nc.tensor.transpose(tr[:], at_tiles[(0, j)][:],
                                                  iden_sb[:])
                              nc.vector.tensor_copy(attnT[0][:, qsl2], tr[:])

                      closures = [lambda: pv_norm(0, pva), lambda: pv_norm(1, pvb)]
                      if last and pair == 0:
                          closures.append(early_tr)
                      if last and pair == 1:
                          for p in pvq:  # drain everything: no next loop
                              p()
                          pvq.clear()
                          for p in closures:
                              p()
                      else:
                          pvq.extend(closures)
                  return at_tiles

              # fused driver: projection tile t feeds attention q-tile t
              # (causal: qt needs only k <= (t+1)*512). The NEXT projection
              # tile and the PREVIOUS qt's transposes/out-projection are
              # sliced into attention's kb loop as fill work so PE never
              # starves during exp waits.
              tailp = []
              pvq = []
              for t in range(SB):
                  up, vp = proj_pieces(t)
                  for p in up:
                      p()
                  if t == 0:
                      for p in vp:  # qt0 has no off-diagonal kbs to fill
                          p()
                      vp = []
                  # V chains must all be emitted before the diagonal kbs
                  # (their PV consumers); tail pieces have no deadline
                  fill = vp + tailp
                  at_tiles = emit_attn(t, fill, pvq, last=(t == SB - 1))
                  for p in fill:  # flush any leftover fill work
                      p()
                  tailp = tail_pieces(t, at_tiles, last=(t == SB - 1))
              for p in tailp:
                  p()

    nc.compile()
    return nc


def _host_tables(token_positions):
    pos = np.asarray(token_positions, dtype=np.float32)  # [S]
    half = DK // 2
    freq = THETA ** (-np.arange(0, DK, 2, dtype=np.float32) / DK)  # [32]
    # per-partition tables on [dk(128 = 2 heads of 64), s]
    f64 = np.repeat(freq, 2)          # [64] freq per feature index
    ang64 = pos[None, :] * f64[:, None]  # [64, S]
    cos64 = np.cos(ang64)
    sin64 = np.sin(ang64)
    sign = np.where(np.arange(DK) % 2 == 0, 1.0, -1.0).astype(np.float32)  # +s even, -s odd
    sins64 = sin64 * sign[:, None]
    cosf = np.concatenate([cos64, cos64], axis=0).astype(np.float32)   # [128, S]
    sins = np.concatenate([sins64, sins64], axis=0).astype(np.float32)  # [128, S]
    return cosf, sins


def kernel(x, Wq, Wk, Wv, Wo, token_positions):
    x = np.asarray(x, dtype=np.float32)
    Wq = np.asarray(Wq, dtype=np.float32)
    Wk = np.asarray(Wk, dtype=np.float32)
    Wv = np.asarray(Wv, dtype=np.float32)
    Wo = np.asarray(Wo, dtype=np.float32)

    if "nc" not in _CACHED:
        _CACHED["nc"] = _build_nc(iters=int(os.environ.get("BENCH_ITERS", "1")))
    nc = _CACHED["nc"]

    cosf, sins = _host_tables(token_positions)
    cosf = cosf.astype(np.float16)
    sins = sins.astype(np.float16)
    tri = np.triu(np.ones((128, 128), dtype=np.float16))  # tri[k, j] = 1 if j >= k
    iden = np.eye(128, dtype=np.float16)

    xT = [np.ascontiguousarray(x[b].T).astype(np.float16) for b in range(B)]
    in_maps = []
    for c in range(NCORES):
        b, g = c // GROUPS, c % GROUPS
        R = slice(g * GF, (g + 1) * GF)
        wqkT = np.ascontiguousarray(
            np.concatenate([Wq[R].T, Wk[R].T], axis=1)).astype(np.float16)  # [D, 512]
        wvT = np.ascontiguousarray(Wv[R].T).astype(np.float16)              # [D, 256]
        woT = np.ascontiguousarray(Wo[:, R].T).astype(np.float16)  # [256, D]
        in_maps.append({
            "xT": xT[b], "wqkT": wqkT, "wvT": wvT, "woT": woT,
            "cosf": cosf, "sins": sins, "tri": tri, "iden": iden,
            "cachebust": np.zeros((int(os.environ.get("BENCH_ITERS", "1")), KVER), dtype=np.float32),
        })

    try:
        res = run_bass_kernel_spmd(nc, in_maps, core_ids=list(range(NCORES)))
    except Exception:
        # transient NRT_EXEC_UNIT_UNRECOVERABLE flakes recover on retry
        import time as _time
        _time.sleep(2.0)
        res = run_bass_kernel_spmd(nc, in_maps, core_ids=list(range(NCORES)))
    _CACHED["last_results"] = res
    outs = [r["out"] for r in res.results]  # each [S, D] partial
    full = np.empty((B, S, D), dtype=np.float32)
    for b in range(B):
        full[b] = sum(outs[b * GROUPS + g] for g in range(GROUPS))
    return full
